# revision 25
# baseline (speedup 1.0000x reference)
"""Trainium2 Bass kernel for nn_DynamicAdapter (dense-MoE adapter block).

Math (per reference), after merging the second expert matmul with the fused
projection — (h1 @ W2 + b2) @ Wf == h1 @ (W2 @ Wf) + b2 @ Wf — which halves
the dominant FLOPs:
  pooled = mean_s(hidden)                               [B, H]
  gate = softmax(MLP_sel(MLP_ana(pooled)))              [B, E]
  h1_e = gelu(x @ W1_e + b1_e) * g_be                   [T, H/2]
  fused = sum_e h1_e @ W2f_e + (gate @ b2f + f_b) + x
  out = layernorm(fused) * ln_g + ln_b
with W2f_e = W2_e @ Wf_e  [H/2, H],  b2f_e = b2_e @ Wf_e  [H].

Sharding: token-parallel, no collectives. Core c handles tokens
{(b, c*256+j)} — 1024 tokens; every core runs all 16 experts on its tokens
(weights replicated) and computes the tiny gate MLP redundantly from the
host-prepared pooled mean (a [4,1024] reduction done once on the host during
input sharding — streaming it beats an 8-core AllReduce by ~80us).

Structure: two 512-token chunks. Per chunk, an mm1 phase computes gate-scaled
h1 for all 16 experts (PE: fp8 DoubleRow matmuls; ACT: gelu to bf16; DVE:
gate-scale to fp8), then one proj pass accumulates all 16 experts into 8 PSUM
banks (4 token-tiles x 2 H-halves) so the fp32 drain happens once per output
tile, flowing straight into the residual + layernorm tail. PSUM pools for the
mm1/proj phases alternate via alloc/release. DMA descriptor generation is
spread across the sync (x, W1, residual, out), scalar (W2f), and gpsimd
(gate weights) queues so no single queue serializes the streams.

Precision: all big matmuls fp8e4 with DoubleRow (2 contraction elems/cycle).
W1 and W2f are pre-scaled by 64 so their values sit in fp8e4's normal range;
h1 carries gate*16 so it stays in normals too; the 1/(64*16) is folded into
the PSUM drain. PSUM accumulation is fp32, and the residual + layernorm tail
runs in fp32, so fp8 noise only perturbs the small MoE delta (~2% of the
residual magnitude).
"""

import numpy as np
import ml_dtypes

import concourse.bacc as bacc
import concourse.mybir as mybir
import concourse.tile as tile
from concourse import bass_utils

BF16 = ml_dtypes.bfloat16
FP8 = ml_dtypes.float8_e4m3fn

B, S, H, E = 4, 2048, 1024, 16
NCORES = 8
P = 128
TOK = B * S            # 8192 tokens total
TPC = TOK // NCORES    # 1024 tokens per core
SC = S // NCORES       # 256 tokens per (batch, core)
HT = H // P            # 8 h-tiles
F1 = H // 2            # 512 expert hidden
F1T = F1 // P          # 4 f1-tiles
TCH = 512              # token chunk (2 batch-chunks)
NCH = TPC // TCH       # 2 chunks
TT = TPC // P          # 8 token-tiles
WSCALE = 64.0          # fp8 pre-scale on W1 / W2f
HSCALE = 16.0          # gate pre-scale applied to h1 (keeps h1*g in fp8 normals)
FBB_AT = 4             # defer gate-dependent PE ops past this many experts

dt8 = mybir.dt.float8e4
dt16 = mybir.dt.bfloat16
dt32 = mybir.dt.float32
AF = mybir.ActivationFunctionType
ALU = mybir.AluOpType
AX = mybir.AxisListType
DR = mybir.MatmulPerfMode.DoubleRow

_BUILT = {}


def _build(reps=1):
    if reps in _BUILT:
        return _BUILT[reps]

    nc = bacc.Bacc("TRN2", target_bir_lowering=False, debug=False)

    # ---- kernel I/O ----
    xsh = nc.dram_tensor("xsh", [HT, P, TPC], dt8, kind="ExternalInput").ap()
    xres = nc.dram_tensor("xres", [TPC, H], dt32, kind="ExternalInput").ap()
    w1 = nc.dram_tensor("w1", [E, HT, P, F1], dt8, kind="ExternalInput").ap()
    w2f = nc.dram_tensor("w2f", [E, F1T, P, H], dt8, kind="ExternalInput").ap()
    b1d = nc.dram_tensor("b1d", [P, E * F1T], dt32, kind="ExternalInput").ap()
    b2fd = nc.dram_tensor("b2fd", [E, H], dt16, kind="ExternalInput").ap()
    pooled_d = nc.dram_tensor("pooled_d", [P, HT * B], dt16, kind="ExternalInput").ap()
    a1 = nc.dram_tensor("a1", [HT, P, F1], dt16, kind="ExternalInput").ap()
    a2 = nc.dram_tensor("a2", [4, P, 256], dt16, kind="ExternalInput").ap()
    a3 = nc.dram_tensor("a3", [2, P, 128], dt16, kind="ExternalInput").ap()
    s1 = nc.dram_tensor("s1", [P, 64], dt16, kind="ExternalInput").ap()
    s2 = nc.dram_tensor("s2", [64, 32], dt16, kind="ExternalInput").ap()
    s3 = nc.dram_tensor("s3", [32, 16], dt16, kind="ExternalInput").ap()
    ab1 = nc.dram_tensor("ab1", [P, 4], dt32, kind="ExternalInput").ap()
    ab2 = nc.dram_tensor("ab2", [P, 2], dt32, kind="ExternalInput").ap()
    ab3 = nc.dram_tensor("ab3", [P, 1], dt32, kind="ExternalInput").ap()
    sb1 = nc.dram_tensor("sb1", [64, 1], dt32, kind="ExternalInput").ap()
    sb2 = nc.dram_tensor("sb2", [32, 1], dt32, kind="ExternalInput").ap()
    sb3 = nc.dram_tensor("sb3", [B, E], dt32, kind="ExternalInput").ap()
    id128 = nc.dram_tensor("id128", [P, P], dt32, kind="ExternalInput").ap()
    fbbc_d = nc.dram_tensor("fbbc_d", [P, H], dt32, kind="ExternalInput").ap()
    gbc_d = nc.dram_tensor("gbc_d", [P, H], dt32, kind="ExternalInput").ap()
    bbc_d = nc.dram_tensor("bbc_d", [P, H], dt32, kind="ExternalInput").ap()
    out = nc.dram_tensor("out", [TPC, H], dt32, kind="ExternalOutput").ap()

    env = locals()
    with tile.TileContext(nc) as tc:
        for _ in range(reps):
            _emit(tc, env)
    nc.compile()
    _BUILT[reps] = nc
    return nc


def _emit(tc, t):
    nc = tc.nc
    with (
        tc.tile_pool(name="persist", bufs=1) as pp,
        tc.tile_pool(name="w1pool", bufs=3) as w1p,
        tc.tile_pool(name="w2pool", bufs=2) as w2p,
        tc.tile_pool(name="htmp", bufs=3) as hp,
        tc.tile_pool(name="hall", bufs=2) as hap,
    ):
        # ---------- critical-path DMAs first: x shard + expert-0 W1 ----------
        xs = pp.tile([P, HT, TPC], dt8, name="xs", tag="xs")
        src = t["xsh"].rearrange("j p t -> p j t")
        nc.sync.dma_start(out=xs[:, 0:4, :], in_=src[:, 0:4, :])
        nc.sync.dma_start(out=xs[:, 4:8, :], in_=src[:, 4:8, :])

        def fetch_w1(ch, e):
            w1t = w1p.tile([P, HT, F1], dt8, name=f"w1t{ch}_{e}", tag="w1t")
            src1 = t["w1"][e].rearrange("i p f -> p i f")
            nc.sync.dma_start(out=w1t[:, 0:4, :], in_=src1[:, 0:4, :])
            nc.sync.dma_start(out=w1t[:, 4:8, :], in_=src1[:, 4:8, :])
            return w1t

        def fetch_w2(e):
            # W2f streamed on the scalar queue: it is consumed during proj
            # phases, when ACT is otherwise idle (gelus run in mm1 phases).
            w2t = w2p.tile([P, F1T, H], dt8, name=f"w2t{e}", tag="w2t")
            src2 = t["w2f"][e].rearrange("m p h -> p m h")
            nc.scalar.dma_start(out=w2t[:, 0:2, :], in_=src2[:, 0:2, :])
            nc.scalar.dma_start(out=w2t[:, 2:4, :], in_=src2[:, 2:4, :])
            return w2t

        w1_cache = {(0, 0): fetch_w1(0, 0)}
        b1_sb = pp.tile([P, E * F1T], dt32, name="b1_sb", tag="b1_sb")
        nc.sync.dma_start(out=b1_sb[:, :], in_=t["b1d"][:, :])
        eps = pp.tile([P, 1], dt32, name="eps", tag="eps")
        nc.vector.memset(eps[:, :], 1e-5)

        # per-(b,e) gate broadcast to all partitions, pre-scaled by HSCALE
        gscH = pp.tile([P, B * E], dt32, name="gscH", tag="gscH")
        # drain scale constant: 1 / (WSCALE * HSCALE)
        hscl = pp.tile([P, 1], dt32, name="hscl", tag="hscl")
        nc.vector.memset(hscl[:, :], 1.0 / (WSCALE * HSCALE))
        # per-batch f_b + gate @ b2f broadcast tiles (tail bias)
        fbb = [
            pp.tile([P, H], dt32, name=f"fbb{b}", tag=f"fbb{b}") for b in range(B)
        ]
        # ---------- phase 0: gate MLP from host-pooled input ----------
        gw = tc.alloc_tile_pool(name="gw", bufs=1)
        psgp = tc.alloc_tile_pool(name="psgp", bufs=2, space="PSUM")
        psfp = tc.alloc_tile_pool(name="psfp", bufs=2, space="PSUM")

        # gate-MLP weight DMAs on the gpsimd queue (idle early); ordered so
        # the first MLP layers' operands land first.
        pooledt = gw.tile([P, HT, B], dt16, name="pooledt", tag="pooledt")
        nc.gpsimd.dma_start(
            out=pooledt[:, :, :], in_=t["pooled_d"].rearrange("p (i b) -> p i b", b=B)
        )
        a1_sb = gw.tile([P, HT, F1], dt16, name="a1_sb", tag="a1_sb")
        nc.gpsimd.dma_start(out=a1_sb[:, :, :], in_=t["a1"].rearrange("i p f -> p i f"))
        ab1_sb = gw.tile([P, 4], dt32, name="ab1_sb", tag="ab1_sb")
        nc.gpsimd.dma_start(out=ab1_sb[:, :], in_=t["ab1"][:, :])
        a2_sb = gw.tile([P, 4, 256], dt16, name="a2_sb", tag="a2_sb")
        nc.gpsimd.dma_start(out=a2_sb[:, :, :], in_=t["a2"].rearrange("i p f -> p i f"))
        ab2_sb = gw.tile([P, 2], dt32, name="ab2_sb", tag="ab2_sb")
        nc.gpsimd.dma_start(out=ab2_sb[:, :], in_=t["ab2"][:, :])
        a3_sb = gw.tile([P, 2, 128], dt16, name="a3_sb", tag="a3_sb")
        nc.gpsimd.dma_start(out=a3_sb[:, :, :], in_=t["a3"].rearrange("i p f -> p i f"))
        ab3_sb = gw.tile([P, 1], dt32, name="ab3_sb", tag="ab3_sb")
        nc.gpsimd.dma_start(out=ab3_sb[:, :], in_=t["ab3"][:, :])
        s1_sb = gw.tile([P, 64], dt16, name="s1_sb", tag="s1_sb")
        nc.gpsimd.dma_start(out=s1_sb[:, :], in_=t["s1"][:, :])
        sb1_sb = gw.tile([64, 1], dt32, name="sb1_sb", tag="sb1_sb")
        nc.gpsimd.dma_start(out=sb1_sb[:, :], in_=t["sb1"][:, :])
        s2_sb = gw.tile([64, 32], dt16, name="s2_sb", tag="s2_sb")
        nc.gpsimd.dma_start(out=s2_sb[:, :], in_=t["s2"][:, :])
        sb2_sb = gw.tile([32, 1], dt32, name="sb2_sb", tag="sb2_sb")
        nc.gpsimd.dma_start(out=sb2_sb[:, :], in_=t["sb2"][:, :])
        s3_sb = gw.tile([32, 16], dt16, name="s3_sb", tag="s3_sb")
        nc.gpsimd.dma_start(out=s3_sb[:, :], in_=t["s3"][:, :])
        sb3_sb = gw.tile([B, E], dt32, name="sb3_sb", tag="sb3_sb")
        nc.gpsimd.dma_start(out=sb3_sb[:, :], in_=t["sb3"][:, :])
        b2f_sb = gw.tile([E, H], dt16, name="b2f_sb", tag="b2f_sb")
        nc.gpsimd.dma_start(out=b2f_sb[:, :], in_=t["b2fd"][:, :])
        id_sb = gw.tile([P, P], dt32, name="id_sb", tag="id_sb")
        nc.gpsimd.dma_start(out=id_sb[:, :], in_=t["id128"][:, :])
        # tail constants last on the gpsimd queue (needed late)
        fbbc = pp.tile([P, H], dt32, name="fbbc", tag="fbbc")
        nc.gpsimd.dma_start(out=fbbc[:, :], in_=t["fbbc_d"][:, :])
        gbc = pp.tile([P, H], dt32, name="gbc", tag="gbc")
        nc.gpsimd.dma_start(out=gbc[:, :], in_=t["gbc_d"][:, :])
        bbc = pp.tile([P, H], dt32, name="bbc", tag="bbc")
        nc.gpsimd.dma_start(out=bbc[:, :], in_=t["bbc_d"][:, :])

        gate_bc = gw.tile([P, B * E], dt32, name="gate_bc", tag="gate_bc")

        def emit_gate():
            t1 = gw.tile([P, 16], dt16, name="t1", tag="t1")
            for m in range(4):
                psg = psgp.tile([P, B], dt32, name="psg1", tag="psg")
                for i in range(HT):
                    nc.tensor.matmul(
                        psg[:, :], a1_sb[:, i, m * P : (m + 1) * P], pooledt[:, i, :],
                        start=(i == 0), stop=(i == HT - 1),
                    )
                nc.scalar.activation(
                    t1[:, m * B : (m + 1) * B], psg[:, :], AF.Gelu,
                    bias=ab1_sb[:, m : m + 1],
                )
            t2 = gw.tile([P, 8], dt16, name="t2", tag="t2")
            for m in range(2):
                psg = psgp.tile([P, B], dt32, name="psg2", tag="psg")
                for i in range(4):
                    nc.tensor.matmul(
                        psg[:, :], a2_sb[:, i, m * P : (m + 1) * P],
                        t1[:, i * B : (i + 1) * B],
                        start=(i == 0), stop=(i == 3),
                    )
                nc.scalar.activation(
                    t2[:, m * B : (m + 1) * B], psg[:, :], AF.Gelu,
                    bias=ab2_sb[:, m : m + 1],
                )
            t3 = gw.tile([P, B], dt16, name="t3", tag="t3")
            psg = psgp.tile([P, B], dt32, name="psg3", tag="psg")
            for i in range(2):
                nc.tensor.matmul(
                    psg[:, :], a3_sb[:, i, :], t2[:, i * B : (i + 1) * B],
                    start=(i == 0), stop=(i == 1),
                )
            nc.scalar.activation(t3[:, :], psg[:, :], AF.Identity, bias=ab3_sb[:, 0:1])

            g1 = gw.tile([64, B], dt16, name="g1", tag="g1")
            psg = psgp.tile([64, B], dt32, name="psg4", tag="psg")
            nc.tensor.matmul(psg[:, :], s1_sb[:, :], t3[:, :], start=True, stop=True)
            nc.scalar.activation(g1[:, :], psg[:, :], AF.Gelu, bias=sb1_sb[:, 0:1])

            g2 = gw.tile([32, B], dt16, name="g2", tag="g2")
            psg = psgp.tile([32, B], dt32, name="psg5", tag="psg")
            nc.tensor.matmul(psg[:, :], s2_sb[:, :], g1[:, :], start=True, stop=True)
            nc.scalar.activation(g2[:, :], psg[:, :], AF.Gelu, bias=sb2_sb[:, 0:1])

            # flip to token-major: z[b, e]
            z = gw.tile([B, E], dt32, name="z", tag="z")
            psg = psgp.tile([B, E], dt32, name="psg6", tag="psg")
            nc.tensor.matmul(psg[:, :], g2[:, :], s3_sb[:, :], start=True, stop=True)
            nc.scalar.copy(z[:, :], psg[:, :])
            nc.vector.tensor_add(z[:, :], z[:, :], sb3_sb[:, :])

            # softmax over E (free dim)
            mx = gw.tile([B, 1], dt32, name="mx", tag="mx")
            nc.vector.reduce_max(mx[:, :], z[:, :], axis=AX.X)
            nc.vector.tensor_scalar_sub(z[:, :], z[:, :], mx[:, 0:1])
            sums = gw.tile([B, 1], dt32, name="sums", tag="sums")
            exps = gw.tile([B, E], dt32, name="exps", tag="exps")
            nc.scalar.activation(exps[:, :], z[:, :], AF.Exp, accum_out=sums[:, 0:1])
            rinv = gw.tile([B, 1], dt32, name="rinv", tag="rinv")
            nc.vector.reciprocal(rinv[:, :], sums[:, :])
            gate4 = gw.tile([B, E], dt32, name="gate4", tag="gate4")
            nc.vector.tensor_scalar_mul(gate4[:, :], exps[:, :], rinv[:, 0:1])

            # broadcast gate to all 128 partitions via DRAM bounce
            dp = tc.alloc_tile_pool(name="dramp", bufs=1, space="DRAM")
            gsc = dp.tile([1, B * E], dt32, name="gsc", tag="gsc")
            nc.sync.dma_start(
                out=gsc.rearrange("o (b e) -> (o b) e", b=B), in_=gate4[:, :]
            )
            gflat = gw.tile([1, B * E], dt32, name="gflat", tag="gflat")
            nc.sync.dma_start(out=gflat[:, :], in_=gsc[:, :])
            dp.release()
            nc.gpsimd.partition_broadcast(gate_bc[:, :], gflat[:, :])
            nc.scalar.mul(gscH[:, :], gate_bc[:, :], HSCALE)

        def emit_fbb():
            # fbb[b] = f_b + sum_e gate[b,e] * b2f[e] broadcast to 128 parts:
            # transpose gate_bc slice -> [E, P] (all cols equal), then matmul
            # with b2f so the output partition dim is already broadcast.
            # Emitted a couple of experts into mm1 so the PE never waits on
            # the gate broadcast round trip.
            for b in range(B):
                psT = psgp.tile([E, P], dt32, name=f"psT{b}", tag="psg")
                nc.tensor.transpose(
                    psT[:, :], gate_bc[:, b * E : (b + 1) * E], id_sb[:, :]
                )
                gbT = gw.tile([E, P], dt16, name=f"gbT{b}", tag=f"gbT{b}")
                nc.scalar.copy(gbT[:, :], psT[:, :])
                for n in range(2):
                    psF = psfp.tile([P, 512], dt32, name="psF", tag="psF")
                    nc.tensor.matmul(
                        psF[:, :], gbT[:, :], b2f_sb[:, n * 512 : (n + 1) * 512],
                        start=True, stop=True,
                    )
                    nc.vector.tensor_add(
                        fbb[b][:, n * 512 : (n + 1) * 512],
                        psF[:, :],
                        fbbc[:, n * 512 : (n + 1) * 512],
                    )

        emit_gate()

        # ---------- mm1 phase: gate-scaled h1 for all experts, one chunk ----
        def mm1_phase(ch, ps1p):
            c0 = ch * TCH
            h1a = hap.tile([P, E * F1T, TCH], dt8, name=f"hall{ch}", tag="hall")
            for e in range(E):
                k = (ch, e)
                w1t = w1_cache.pop(k) if k in w1_cache else fetch_w1(ch, e)
                h1t = hp.tile([P, F1T, TCH], dt16, name=f"h1t{ch}_{e}", tag="h1t")
                for m in range(F1T):
                    ps = ps1p.tile([P, TCH], dt32, name="ps1", tag="ps1")
                    for j in range(HT // 2):
                        nc.tensor.matmul(
                            ps[:, :],
                            w1t[:, 2 * j : 2 * j + 2, m * P : (m + 1) * P],
                            xs[:, 2 * j : 2 * j + 2, c0 : c0 + TCH],
                            start=(j == 0), stop=(j == HT // 2 - 1),
                            perf_mode=DR,
                        )
                    nc.scalar.activation(
                        h1t[:, m, :], ps[:, :], AF.Gelu,
                        bias=b1_sb[:, e * F1T + m : e * F1T + m + 1],
                        scale=1.0 / WSCALE,
                    )
                for bh in range(2):
                    gi = (ch * 2 + bh) * E + e
                    for m in range(F1T):
                        nc.vector.tensor_scalar_mul(
                            h1a[:, e * F1T + m, bh * SC : (bh + 1) * SC],
                            h1t[:, m, bh * SC : (bh + 1) * SC],
                            gscH[:, gi : gi + 1],
                        )
                if ch == 0 and e == FBB_AT:
                    emit_fbb()
            return h1a

        # ---------- mm1 chunk 0 (gate/fbb pools still alive) ----------
        ps1p = tc.alloc_tile_pool(name="ps1a", bufs=2, space="PSUM")
        h1a0 = mm1_phase(0, ps1p)
        ps1p.release()
        psfp.release()
        psgp.release()
        gw.release()

        # ---------- tail pools (live across both proj passes) ----------
        txf = tc.alloc_tile_pool(name="txf", bufs=3)
        f2p = tc.alloc_tile_pool(name="f2p", bufs=2)
        tp = tc.alloc_tile_pool(name="tail", bufs=2)
        otp = tc.alloc_tile_pool(name="otp", bufs=2)
        sqp = tc.alloc_tile_pool(name="sqp", bufs=1)

        def emit_tail(tau, f2):
            ssum = tp.tile([P, 1], dt32, name="ssum", tag="ssum")
            nc.vector.reduce_sum(ssum[:, :], f2[:, :], axis=AX.X)
            negmu = tp.tile([P, 1], dt32, name="negmu", tag="negmu")
            nc.vector.tensor_scalar_mul(negmu[:, :], ssum[:, :], -1.0 / H)
            nc.scalar.activation(f2[:, :], f2[:, :], AF.Identity, bias=negmu[:, 0:1])
            sq = sqp.tile([P, H], dt16, name="sq", tag="sq")
            ssq = tp.tile([P, 1], dt32, name="ssq", tag="ssq")
            nc.scalar.activation(sq[:, :], f2[:, :], AF.Square, accum_out=ssq[:, 0:1])
            stdv = tp.tile([P, 1], dt32, name="stdv", tag="stdv")
            nc.scalar.activation(
                stdv[:, :], ssq[:, :], AF.Sqrt, scale=1.0 / H, bias=eps[:, 0:1]
            )
            rinv2 = tp.tile([P, 1], dt32, name="rinv2", tag="rinv2")
            nc.vector.reciprocal(rinv2[:, :], stdv[:, :])
            ot = otp.tile([P, H], dt32, name="ot", tag="ot")
            nc.vector.scalar_tensor_tensor(
                ot[:, :], f2[:, :], rinv2[:, 0:1], gbc[:, :],
                op0=ALU.mult, op1=ALU.mult,
            )
            nc.gpsimd.tensor_add(ot[:, :], ot[:, :], bbc[:, :])
            nc.gpsimd.dma_start(out=t["out"][tau * P : (tau + 1) * P, :], in_=ot[:, :])

        # ---------- proj pass: all experts into PSUM, grouped token-tiles ---
        # Each group of token-tiles accumulates all 16 experts into
        # len(group)*2 PSUM banks, then drains straight into the tail. The
        # final chunk uses two groups so half its tails overlap matmuls.
        def proj_phase(ch, h1a, ps3p, groups):
            xrfs = {}
            for tl in range(4):
                tau = ch * 4 + tl
                xrf = txf.tile([P, H], dt32, name=f"xrf{tau}", tag="xrf")
                nc.sync.dma_start(
                    out=xrf[:, :], in_=t["xres"][tau * P : (tau + 1) * P, :]
                )
                nc.vector.tensor_add(xrf[:, :], xrf[:, :], fbb[tau // 2][:, :])
                xrfs[tau] = xrf
            for gi, group in enumerate(groups):
                banks = {}
                for tl in group:
                    for n in range(2):
                        banks[(tl, n)] = ps3p.tile(
                            [P, TCH], dt32, name=f"ps3_{ch}_{tl}_{n}", tag="ps3"
                        )
                for e in range(E):
                    w2t = fetch_w2(e)
                    for tl in group:
                        toff = tl * P
                        for j in range(F1T // 2):
                            for n in range(2):
                                nc.tensor.matmul(
                                    banks[(tl, n)][:, :],
                                    h1a[:, e * F1T + 2 * j : e * F1T + 2 * j + 2,
                                        toff : toff + P],
                                    w2t[:, 2 * j : 2 * j + 2, n * TCH : (n + 1) * TCH],
                                    start=(e == 0 and j == 0),
                                    stop=(e == E - 1 and j == F1T // 2 - 1),
                                    perf_mode=DR,
                                )
                for tl in group:
                    tau = ch * 4 + tl
                    f2 = f2p.tile([P, H], dt32, name=f"f2_{tau}", tag="f2")
                    for n in range(2):
                        nc.vector.scalar_tensor_tensor(
                            f2[:, n * TCH : (n + 1) * TCH],
                            banks[(tl, n)][:, :],
                            hscl[:, 0:1],
                            xrfs[tau][:, n * TCH : (n + 1) * TCH],
                            op0=ALU.mult, op1=ALU.add,
                        )
                    emit_tail(tau, f2)

        # ---------- main schedule: PSUM pools ping-pong between phases ------
        # prefetch chunk-1 W1 for the first experts so the next mm1 phase
        # starts without waiting on the DMA queue
        w1_cache[(1, 0)] = fetch_w1(1, 0)
        ps3p = tc.alloc_tile_pool(name="ps3a", bufs=8, space="PSUM")
        proj_phase(0, h1a0, ps3p, [(0, 1, 2, 3)])
        ps3p.release()
        ps1p = tc.alloc_tile_pool(name="ps1b", bufs=2, space="PSUM")
        h1a1 = mm1_phase(1, ps1p)
        ps1p.release()
        ps3p = tc.alloc_tile_pool(name="ps3b", bufs=8, space="PSUM")
        proj_phase(1, h1a1, ps3p, [(0, 1), (2, 3)])
        ps3p.release()

        sqp.release()
        otp.release()
        tp.release()
        f2p.release()
        txf.release()


def _prep_inputs(inputs):
    """Host-side sharding/layout prep. Returns per-core input maps."""
    f32 = np.float32

    def bf(x):
        return np.ascontiguousarray(np.asarray(x, dtype=f32)).astype(BF16)

    hs = np.ascontiguousarray(np.asarray(inputs["hidden_states"], dtype=f32))  # [B,S,H]

    e1_w = np.asarray(inputs["e1_w"], f32)          # [E, H, F1]
    e1_b = np.asarray(inputs["e1_b"], f32)          # [E, F1]
    e2_w = np.asarray(inputs["e2_w"], f32)          # [E, F1, H]
    e2_b = np.asarray(inputs["e2_b"], f32)          # [E, H]
    f_w = np.asarray(inputs["f_w"], f32)            # [E, H, H]

    # merge: W2f = W2 @ Wf   [E, F1, H];   b2f = b2 @ Wf   [E, H]
    w2f = np.matmul(e2_w, f_w)
    b2f = np.einsum("eh,ehg->eg", e2_b, f_w)

    # pooled mean over the sequence (gate MLP input), feature-major
    pooled = hs.mean(axis=1)                        # [B, H]
    pooled_t = np.ascontiguousarray(
        pooled.reshape(B, HT, P).transpose(2, 1, 0)
    ).reshape(P, HT * B)

    common = {
        "w1": (e1_w * WSCALE).astype(FP8).reshape(E, HT, P, F1),
        "w2f": (w2f * WSCALE).astype(FP8).reshape(E, F1T, P, H),
        "b1d": np.ascontiguousarray(e1_b.reshape(E, F1T, P).transpose(2, 0, 1)).reshape(P, E * F1T),
        "b2fd": bf(b2f),
        "pooled_d": pooled_t.astype(BF16),
        "a1": bf(inputs["a1_w"]).reshape(HT, P, F1),
        "a2": bf(inputs["a2_w"]).reshape(4, P, 256),
        "a3": bf(inputs["a3_w"]).reshape(2, P, 128),
        "s1": bf(inputs["s1_w"]),
        "s2": bf(inputs["s2_w"]),
        "s3": bf(inputs["s3_w"]),
        "ab1": np.ascontiguousarray(np.asarray(inputs["a1_b"], f32).reshape(4, P).T),
        "ab2": np.ascontiguousarray(np.asarray(inputs["a2_b"], f32).reshape(2, P).T),
        "ab3": np.ascontiguousarray(np.asarray(inputs["a3_b"], f32).reshape(1, P).T),
        "sb1": np.ascontiguousarray(np.asarray(inputs["s1_b"], f32).reshape(64, 1)),
        "sb2": np.ascontiguousarray(np.asarray(inputs["s2_b"], f32).reshape(32, 1)),
        "sb3": np.ascontiguousarray(np.broadcast_to(np.asarray(inputs["s3_b"], f32), (B, E))),
        "id128": np.eye(P, dtype=f32),
        "fbbc_d": np.ascontiguousarray(np.broadcast_to(np.asarray(inputs["f_b"], f32), (P, H))),
        "gbc_d": np.ascontiguousarray(np.broadcast_to(np.asarray(inputs["ln_g"], f32), (P, H))),
        "bbc_d": np.ascontiguousarray(np.broadcast_to(np.asarray(inputs["ln_b"], f32), (P, H))),
    }

    in_maps = []
    for c in range(NCORES):
        shard = hs[:, c * SC : (c + 1) * SC, :]                     # [B, SC, H]
        xsh_c = np.ascontiguousarray(shard.transpose(2, 0, 1)).reshape(H, TPC)
        m = dict(common)
        m["xsh"] = xsh_c.astype(FP8).reshape(HT, P, TPC)
        m["xres"] = np.ascontiguousarray(shard).reshape(TPC, H)
        in_maps.append(m)
    return in_maps


def kernel(**inputs) -> np.ndarray:
    nc = _build()
    in_maps = _prep_inputs(inputs)
    res = bass_utils.run_bass_kernel_spmd(nc, in_maps, core_ids=list(range(NCORES)))
    out_full = np.empty((B, S, H), dtype=np.float32)
    for c in range(NCORES):
        out_full[:, c * SC : (c + 1) * SC, :] = res.results[c]["out"].reshape(B, SC, H)
    return out_full


# revision 36
# speedup vs baseline: 1.0772x; 1.0772x over previous
"""Trainium2 Bass kernel for nn_DynamicAdapter (dense-MoE adapter block).

Math (per reference), after merging the second expert matmul with the fused
projection — (h1 @ W2 + b2) @ Wf == h1 @ (W2 @ Wf) + b2 @ Wf — which halves
the dominant FLOPs:
  pooled = mean_s(hidden)                               [B, H]
  gate = softmax(MLP_sel(MLP_ana(pooled)))              [B, E]
  h1_e = gelu(x @ W1_e + b1_e) * g_be                   [T, H/2]
  fused = sum_e h1_e @ W2f_e + (gate @ b2f + f_b) + x
  out = layernorm(fused) * ln_g + ln_b
with W2f_e = W2_e @ Wf_e  [H/2, H],  b2f_e = b2_e @ Wf_e  [H].

Sharding: token-parallel, no collectives. Core c handles tokens
{(b, c*256+j)} — 1024 tokens; every core runs all 16 experts on its tokens
(weights replicated) and computes the tiny gate MLP redundantly from the
host-prepared pooled mean (a [4,1024] reduction done once on the host during
input sharding — streaming it beats an 8-core AllReduce by ~80us).

Structure: two 512-token chunks. Per chunk, an mm1 phase computes gate-scaled
h1 for all 16 experts (PE: fp8 DoubleRow matmuls; ACT: gelu to bf16; DVE:
gate-scale to fp8), then one proj pass accumulates all 16 experts into 8 PSUM
banks (4 token-tiles x 2 H-halves) so the fp32 drain happens once per output
tile, flowing straight into the residual + layernorm tail. PSUM pools for the
mm1/proj phases alternate via alloc/release. DMA descriptor generation is
spread across the sync (x, W1, residual, out), scalar (W2f), and gpsimd
(gate weights) queues so no single queue serializes the streams.

Precision: all big matmuls fp8e4 with DoubleRow (2 contraction elems/cycle).
W1 and W2f are pre-scaled by 64 so their values sit in fp8e4's normal range;
h1 carries gate*16 so it stays in normals too; the 1/(64*16) is folded into
the PSUM drain. PSUM accumulation is fp32, and the residual + layernorm tail
runs in fp32, so fp8 noise only perturbs the small MoE delta (~2% of the
residual magnitude).
"""

import numpy as np
import ml_dtypes

import concourse.bacc as bacc
import concourse.mybir as mybir
import concourse.tile as tile
from concourse import bass_utils

BF16 = ml_dtypes.bfloat16
FP8 = ml_dtypes.float8_e4m3fn

B, S, H, E = 4, 2048, 1024, 16
NCORES = 8
P = 128
TOK = B * S            # 8192 tokens total
TPC = TOK // NCORES    # 1024 tokens per core
SC = S // NCORES       # 256 tokens per (batch, core)
HT = H // P            # 8 h-tiles
F1 = H // 2            # 512 expert hidden
F1T = F1 // P          # 4 f1-tiles
TCH = 512              # token chunk (2 batch-chunks)
NCH = TPC // TCH       # 2 chunks
TT = TPC // P          # 8 token-tiles
WSCALE = 64.0          # fp8 pre-scale on W1 / W2f
HSCALE = 16.0          # gate pre-scale applied to h1 (keeps h1*g in fp8 normals)
FBB_AT = 4             # defer gate-dependent PE ops past this many experts

dt8 = mybir.dt.float8e4
dt16 = mybir.dt.bfloat16
dt32 = mybir.dt.float32
AF = mybir.ActivationFunctionType
ALU = mybir.AluOpType
AX = mybir.AxisListType
DR = mybir.MatmulPerfMode.DoubleRow

_BUILT = {}


def _build(reps=1):
    if reps in _BUILT:
        return _BUILT[reps]

    nc = bacc.Bacc("TRN2", target_bir_lowering=False, debug=False)

    # ---- kernel I/O ----
    xsh = nc.dram_tensor("xsh", [HT, P, TPC], dt8, kind="ExternalInput").ap()
    xres = nc.dram_tensor("xres", [TPC, H], dt32, kind="ExternalInput").ap()
    w1 = nc.dram_tensor("w1", [E, HT, P, F1], dt8, kind="ExternalInput").ap()
    w2f = nc.dram_tensor("w2f", [E, F1T, P, H], dt8, kind="ExternalInput").ap()
    b1d = nc.dram_tensor("b1d", [P, E * F1T], dt32, kind="ExternalInput").ap()
    b2fd = nc.dram_tensor("b2fd", [E, H], dt16, kind="ExternalInput").ap()
    pooled_d = nc.dram_tensor("pooled_d", [P, HT * B], dt16, kind="ExternalInput").ap()
    a1 = nc.dram_tensor("a1", [HT, P, F1], dt16, kind="ExternalInput").ap()
    a2 = nc.dram_tensor("a2", [4, P, 256], dt16, kind="ExternalInput").ap()
    a3 = nc.dram_tensor("a3", [2, P, 128], dt16, kind="ExternalInput").ap()
    s1 = nc.dram_tensor("s1", [P, 64], dt16, kind="ExternalInput").ap()
    s2 = nc.dram_tensor("s2", [64, 32], dt16, kind="ExternalInput").ap()
    s3 = nc.dram_tensor("s3", [32, 16], dt16, kind="ExternalInput").ap()
    ab1 = nc.dram_tensor("ab1", [P, 4], dt32, kind="ExternalInput").ap()
    ab2 = nc.dram_tensor("ab2", [P, 2], dt32, kind="ExternalInput").ap()
    ab3 = nc.dram_tensor("ab3", [P, 1], dt32, kind="ExternalInput").ap()
    sb1 = nc.dram_tensor("sb1", [64, 1], dt32, kind="ExternalInput").ap()
    sb2 = nc.dram_tensor("sb2", [32, 1], dt32, kind="ExternalInput").ap()
    sb3 = nc.dram_tensor("sb3", [B, E], dt32, kind="ExternalInput").ap()
    id128 = nc.dram_tensor("id128", [P, P], dt32, kind="ExternalInput").ap()
    fbbc_d = nc.dram_tensor("fbbc_d", [P, H], dt32, kind="ExternalInput").ap()
    gbc_d = nc.dram_tensor("gbc_d", [P, H], dt32, kind="ExternalInput").ap()
    bbc_d = nc.dram_tensor("bbc_d", [P, H], dt32, kind="ExternalInput").ap()
    out = nc.dram_tensor("out", [TPC, H], dt32, kind="ExternalOutput").ap()

    env = locals()
    with tile.TileContext(nc) as tc:
        for _ in range(reps):
            _emit(tc, env)
    nc.compile()
    _BUILT[reps] = nc
    return nc


def _emit(tc, t):
    nc = tc.nc
    with (
        tc.tile_pool(name="persist", bufs=1) as pp,
        tc.tile_pool(name="w1pool", bufs=3) as w1p,
        tc.tile_pool(name="w2pool", bufs=3) as w2p,
        tc.tile_pool(name="htmp", bufs=3) as hp,
        tc.tile_pool(name="hall", bufs=2) as hap,
        tc.tile_pool(name="ps1p", bufs=2, space="PSUM") as ps1p,
    ):
        # ---------- critical-path DMAs first: x shard + expert-0 W1 ----------
        xs = pp.tile([P, HT, TPC], dt8, name="xs", tag="xs")
        src = t["xsh"].rearrange("j p t -> p j t")
        nc.sync.dma_start(out=xs[:, 0:4, :], in_=src[:, 0:4, :])
        nc.sync.dma_start(out=xs[:, 4:8, :], in_=src[:, 4:8, :])

        def fetch_w1(ch, e):
            w1t = w1p.tile([P, HT, F1], dt8, name=f"w1t{ch}_{e}", tag="w1t")
            src1 = t["w1"][e].rearrange("i p f -> p i f")
            nc.sync.dma_start(out=w1t[:, 0:4, :], in_=src1[:, 0:4, :])
            nc.sync.dma_start(out=w1t[:, 4:8, :], in_=src1[:, 4:8, :])
            return w1t

        _w2seq = [0]

        def fetch_w2(e):
            # W2f streamed on the scalar queue: it is consumed during proj
            # phases, when ACT is mostly idle (gelus run in mm1 phases).
            _w2seq[0] += 1
            w2t = w2p.tile([P, F1T, H], dt8, name=f"w2t{_w2seq[0]}_{e}", tag="w2t")
            src2 = t["w2f"][e].rearrange("m p h -> p m h")
            nc.scalar.dma_start(out=w2t[:, 0:2, :], in_=src2[:, 0:2, :])
            nc.scalar.dma_start(out=w2t[:, 2:4, :], in_=src2[:, 2:4, :])
            return w2t

        w2_next = [None]

        w1_cache = {(0, 0): fetch_w1(0, 0)}
        b1_sb = pp.tile([P, E * F1T], dt32, name="b1_sb", tag="b1_sb")
        nc.sync.dma_start(out=b1_sb[:, :], in_=t["b1d"][:, :])
        eps = pp.tile([P, 1], dt32, name="eps", tag="eps")
        nc.vector.memset(eps[:, :], 1e-5)

        # per-(b,e) gate broadcast to all partitions, pre-scaled by HSCALE
        gscH = pp.tile([P, B * E], dt32, name="gscH", tag="gscH")
        # drain scale constant: 1 / (WSCALE * HSCALE)
        hscl = pp.tile([P, 1], dt32, name="hscl", tag="hscl")
        nc.vector.memset(hscl[:, :], 1.0 / (WSCALE * HSCALE))
        # per-batch f_b + gate @ b2f broadcast tiles (tail bias)
        fbb = [
            pp.tile([P, H], dt32, name=f"fbb{b}", tag=f"fbb{b}") for b in range(B)
        ]
        # ---------- phase 0: gate MLP from host-pooled input ----------
        gw = tc.alloc_tile_pool(name="gw", bufs=1)
        psgp = tc.alloc_tile_pool(name="psgp", bufs=2, space="PSUM")
        psfp = tc.alloc_tile_pool(name="psfp", bufs=2, space="PSUM")

        # gate-MLP weight DMAs on the gpsimd queue (idle early); ordered so
        # the first MLP layers' operands land first.
        pooledt = gw.tile([P, HT, B], dt16, name="pooledt", tag="pooledt")
        nc.gpsimd.dma_start(
            out=pooledt[:, :, :], in_=t["pooled_d"].rearrange("p (i b) -> p i b", b=B)
        )
        a1_sb = gw.tile([P, HT, F1], dt16, name="a1_sb", tag="a1_sb")
        nc.gpsimd.dma_start(out=a1_sb[:, :, :], in_=t["a1"].rearrange("i p f -> p i f"))
        ab1_sb = gw.tile([P, 4], dt32, name="ab1_sb", tag="ab1_sb")
        nc.gpsimd.dma_start(out=ab1_sb[:, :], in_=t["ab1"][:, :])
        a2_sb = gw.tile([P, 4, 256], dt16, name="a2_sb", tag="a2_sb")
        nc.gpsimd.dma_start(out=a2_sb[:, :, :], in_=t["a2"].rearrange("i p f -> p i f"))
        ab2_sb = gw.tile([P, 2], dt32, name="ab2_sb", tag="ab2_sb")
        nc.gpsimd.dma_start(out=ab2_sb[:, :], in_=t["ab2"][:, :])
        a3_sb = gw.tile([P, 2, 128], dt16, name="a3_sb", tag="a3_sb")
        nc.gpsimd.dma_start(out=a3_sb[:, :, :], in_=t["a3"].rearrange("i p f -> p i f"))
        ab3_sb = gw.tile([P, 1], dt32, name="ab3_sb", tag="ab3_sb")
        nc.gpsimd.dma_start(out=ab3_sb[:, :], in_=t["ab3"][:, :])
        s1_sb = gw.tile([P, 64], dt16, name="s1_sb", tag="s1_sb")
        nc.gpsimd.dma_start(out=s1_sb[:, :], in_=t["s1"][:, :])
        sb1_sb = gw.tile([64, 1], dt32, name="sb1_sb", tag="sb1_sb")
        nc.gpsimd.dma_start(out=sb1_sb[:, :], in_=t["sb1"][:, :])
        s2_sb = gw.tile([64, 32], dt16, name="s2_sb", tag="s2_sb")
        nc.gpsimd.dma_start(out=s2_sb[:, :], in_=t["s2"][:, :])
        sb2_sb = gw.tile([32, 1], dt32, name="sb2_sb", tag="sb2_sb")
        nc.gpsimd.dma_start(out=sb2_sb[:, :], in_=t["sb2"][:, :])
        s3_sb = gw.tile([32, 16], dt16, name="s3_sb", tag="s3_sb")
        nc.gpsimd.dma_start(out=s3_sb[:, :], in_=t["s3"][:, :])
        sb3_sb = gw.tile([B, E], dt32, name="sb3_sb", tag="sb3_sb")
        nc.gpsimd.dma_start(out=sb3_sb[:, :], in_=t["sb3"][:, :])
        b2f_sb = gw.tile([E, H], dt16, name="b2f_sb", tag="b2f_sb")
        nc.gpsimd.dma_start(out=b2f_sb[:, :], in_=t["b2fd"][:, :])
        id_sb = gw.tile([P, P], dt32, name="id_sb", tag="id_sb")
        nc.gpsimd.dma_start(out=id_sb[:, :], in_=t["id128"][:, :])
        # tail constants last on the gpsimd queue (needed late)
        fbbc = pp.tile([P, H], dt32, name="fbbc", tag="fbbc")
        nc.gpsimd.dma_start(out=fbbc[:, :], in_=t["fbbc_d"][:, :])
        gbc = pp.tile([P, H], dt32, name="gbc", tag="gbc")
        nc.gpsimd.dma_start(out=gbc[:, :], in_=t["gbc_d"][:, :])
        bbc = pp.tile([P, H], dt32, name="bbc", tag="bbc")
        nc.gpsimd.dma_start(out=bbc[:, :], in_=t["bbc_d"][:, :])

        gate_bc = gw.tile([P, B * E], dt32, name="gate_bc", tag="gate_bc")

        def emit_gate():
            t1 = gw.tile([P, 16], dt16, name="t1", tag="t1")
            for m in range(4):
                psg = psgp.tile([P, B], dt32, name="psg1", tag="psg")
                for i in range(HT):
                    nc.tensor.matmul(
                        psg[:, :], a1_sb[:, i, m * P : (m + 1) * P], pooledt[:, i, :],
                        start=(i == 0), stop=(i == HT - 1),
                    )
                nc.scalar.activation(
                    t1[:, m * B : (m + 1) * B], psg[:, :], AF.Gelu,
                    bias=ab1_sb[:, m : m + 1],
                )
            t2 = gw.tile([P, 8], dt16, name="t2", tag="t2")
            for m in range(2):
                psg = psgp.tile([P, B], dt32, name="psg2", tag="psg")
                for i in range(4):
                    nc.tensor.matmul(
                        psg[:, :], a2_sb[:, i, m * P : (m + 1) * P],
                        t1[:, i * B : (i + 1) * B],
                        start=(i == 0), stop=(i == 3),
                    )
                nc.scalar.activation(
                    t2[:, m * B : (m + 1) * B], psg[:, :], AF.Gelu,
                    bias=ab2_sb[:, m : m + 1],
                )
            t3 = gw.tile([P, B], dt16, name="t3", tag="t3")
            psg = psgp.tile([P, B], dt32, name="psg3", tag="psg")
            for i in range(2):
                nc.tensor.matmul(
                    psg[:, :], a3_sb[:, i, :], t2[:, i * B : (i + 1) * B],
                    start=(i == 0), stop=(i == 1),
                )
            nc.scalar.activation(t3[:, :], psg[:, :], AF.Identity, bias=ab3_sb[:, 0:1])

            g1 = gw.tile([64, B], dt16, name="g1", tag="g1")
            psg = psgp.tile([64, B], dt32, name="psg4", tag="psg")
            nc.tensor.matmul(psg[:, :], s1_sb[:, :], t3[:, :], start=True, stop=True)
            nc.scalar.activation(g1[:, :], psg[:, :], AF.Gelu, bias=sb1_sb[:, 0:1])

            g2 = gw.tile([32, B], dt16, name="g2", tag="g2")
            psg = psgp.tile([32, B], dt32, name="psg5", tag="psg")
            nc.tensor.matmul(psg[:, :], s2_sb[:, :], g1[:, :], start=True, stop=True)
            nc.scalar.activation(g2[:, :], psg[:, :], AF.Gelu, bias=sb2_sb[:, 0:1])

            # flip to token-major: z[b, e]
            z = gw.tile([B, E], dt32, name="z", tag="z")
            psg = psgp.tile([B, E], dt32, name="psg6", tag="psg")
            nc.tensor.matmul(psg[:, :], g2[:, :], s3_sb[:, :], start=True, stop=True)
            nc.scalar.copy(z[:, :], psg[:, :])
            nc.vector.tensor_add(z[:, :], z[:, :], sb3_sb[:, :])

            # softmax over E (free dim)
            mx = gw.tile([B, 1], dt32, name="mx", tag="mx")
            nc.vector.reduce_max(mx[:, :], z[:, :], axis=AX.X)
            nc.vector.tensor_scalar_sub(z[:, :], z[:, :], mx[:, 0:1])
            sums = gw.tile([B, 1], dt32, name="sums", tag="sums")
            exps = gw.tile([B, E], dt32, name="exps", tag="exps")
            nc.scalar.activation(exps[:, :], z[:, :], AF.Exp, accum_out=sums[:, 0:1])
            rinv = gw.tile([B, 1], dt32, name="rinv", tag="rinv")
            nc.vector.reciprocal(rinv[:, :], sums[:, :])
            gate4 = gw.tile([B, E], dt32, name="gate4", tag="gate4")
            nc.vector.tensor_scalar_mul(gate4[:, :], exps[:, :], rinv[:, 0:1])

            # broadcast gate to all 128 partitions via DRAM bounce (gpsimd
            # queue: keeps the sync queue free for W1 descriptor generation)
            dp = tc.alloc_tile_pool(name="dramp", bufs=1, space="DRAM")
            gsc = dp.tile([1, B * E], dt32, name="gsc", tag="gsc")
            nc.gpsimd.dma_start(
                out=gsc.rearrange("o (b e) -> (o b) e", b=B), in_=gate4[:, :]
            )
            gflat = gw.tile([1, B * E], dt32, name="gflat", tag="gflat")
            nc.gpsimd.dma_start(out=gflat[:, :], in_=gsc[:, :])
            dp.release()
            nc.gpsimd.partition_broadcast(gate_bc[:, :], gflat[:, :])
            nc.scalar.mul(gscH[:, :], gate_bc[:, :], HSCALE)

        def emit_fbb():
            # fbb[b] = f_b + sum_e gate[b,e] * b2f[e] broadcast to 128 parts:
            # transpose gate_bc slice -> [E, P] (all cols equal), then matmul
            # with b2f so the output partition dim is already broadcast.
            # Emitted a couple of experts into mm1 so the PE never waits on
            # the gate broadcast round trip.
            for b in range(B):
                psT = psgp.tile([E, P], dt32, name=f"psT{b}", tag="psg")
                nc.tensor.transpose(
                    psT[:, :], gate_bc[:, b * E : (b + 1) * E], id_sb[:, :]
                )
                gbT = gw.tile([E, P], dt16, name=f"gbT{b}", tag=f"gbT{b}")
                nc.scalar.copy(gbT[:, :], psT[:, :])
                for n in range(2):
                    psF = psfp.tile([P, 512], dt32, name="psF", tag="psF")
                    nc.tensor.matmul(
                        psF[:, :], gbT[:, :], b2f_sb[:, n * 512 : (n + 1) * 512],
                        start=True, stop=True,
                    )
                    nc.vector.tensor_add(
                        fbb[b][:, n * 512 : (n + 1) * 512],
                        psF[:, :],
                        fbbc[:, n * 512 : (n + 1) * 512],
                    )

        emit_gate()

        # ---------- mm1 phase: gate-scaled h1 for all experts, one chunk ----
        def mm1_phase(ch):
            c0 = ch * TCH
            h1a = hap.tile([P, E * F1T, TCH], dt8, name=f"hall{ch}", tag="hall")
            for e in range(E):
                k = (ch, e)
                w1t = w1_cache.pop(k) if k in w1_cache else fetch_w1(ch, e)
                h1t = hp.tile([P, F1T, TCH], dt16, name=f"h1t{ch}_{e}", tag="h1t")
                for m in range(F1T):
                    ps = ps1p.tile([P, TCH], dt32, name="ps1", tag="ps1")
                    for j in range(HT // 2):
                        nc.tensor.matmul(
                            ps[:, :],
                            w1t[:, 2 * j : 2 * j + 2, m * P : (m + 1) * P],
                            xs[:, 2 * j : 2 * j + 2, c0 : c0 + TCH],
                            start=(j == 0), stop=(j == HT // 2 - 1),
                            perf_mode=DR,
                        )
                    nc.scalar.activation(
                        h1t[:, m, :], ps[:, :], AF.Gelu,
                        bias=b1_sb[:, e * F1T + m : e * F1T + m + 1],
                        scale=1.0 / WSCALE,
                    )
                for bh in range(2):
                    gi = (ch * 2 + bh) * E + e
                    for m in range(F1T):
                        nc.vector.tensor_scalar_mul(
                            h1a[:, e * F1T + m, bh * SC : (bh + 1) * SC],
                            h1t[:, m, bh * SC : (bh + 1) * SC],
                            gscH[:, gi : gi + 1],
                        )
                if ch == 0 and e == FBB_AT:
                    emit_fbb()
            return h1a

        # ---------- mm1 chunk 0 (gate/fbb pools still alive) ----------
        h1a0 = mm1_phase(0)
        psfp.release()
        psgp.release()
        gw.release()

        # ---------- tail pools (live across both proj passes) ----------
        txf = tc.alloc_tile_pool(name="txf", bufs=3)
        f2p = tc.alloc_tile_pool(name="f2p", bufs=2)
        tp = tc.alloc_tile_pool(name="tail", bufs=2)
        otp = tc.alloc_tile_pool(name="otp", bufs=2)
        sqp = tc.alloc_tile_pool(name="sqp", bufs=1)

        def emit_tail(tau, f2):
            ssum = tp.tile([P, 1], dt32, name="ssum", tag="ssum")
            nc.vector.reduce_sum(ssum[:, :], f2[:, :], axis=AX.X)
            negmu = tp.tile([P, 1], dt32, name="negmu", tag="negmu")
            nc.vector.tensor_scalar_mul(negmu[:, :], ssum[:, :], -1.0 / H)
            nc.scalar.activation(f2[:, :], f2[:, :], AF.Identity, bias=negmu[:, 0:1])
            sq = sqp.tile([P, H], dt16, name="sq", tag="sq")
            ssq = tp.tile([P, 1], dt32, name="ssq", tag="ssq")
            nc.scalar.activation(sq[:, :], f2[:, :], AF.Square, accum_out=ssq[:, 0:1])
            stdv = tp.tile([P, 1], dt32, name="stdv", tag="stdv")
            nc.scalar.activation(
                stdv[:, :], ssq[:, :], AF.Sqrt, scale=1.0 / H, bias=eps[:, 0:1]
            )
            rinv2 = tp.tile([P, 1], dt32, name="rinv2", tag="rinv2")
            nc.vector.reciprocal(rinv2[:, :], stdv[:, :])
            ot = otp.tile([P, H], dt32, name="ot", tag="ot")
            nc.vector.scalar_tensor_tensor(
                ot[:, :], f2[:, :], rinv2[:, 0:1], gbc[:, :],
                op0=ALU.mult, op1=ALU.mult,
            )
            nc.gpsimd.tensor_add(ot[:, :], ot[:, :], bbc[:, :])
            nc.gpsimd.dma_start(out=t["out"][tau * P : (tau + 1) * P, :], in_=ot[:, :])

        # ---------- proj pass: all experts into PSUM, grouped token-tiles ---
        # Each group of token-tiles accumulates all 16 experts into
        # len(group)*2 PSUM banks, then drains straight into the tail. The
        # final chunk uses two groups so half its tails overlap matmuls.
        def proj_phase(ch, h1a, groups, last=False):
            xrfs = {}
            for tl in range(4):
                tau = ch * 4 + tl
                xrf = txf.tile([P, H], dt32, name=f"xrf{tau}", tag="xrf")
                nc.sync.dma_start(
                    out=xrf[:, :], in_=t["xres"][tau * P : (tau + 1) * P, :]
                )
                nc.vector.tensor_add(xrf[:, :], xrf[:, :], fbb[tau // 2][:, :])
                xrfs[tau] = xrf
            for gi, group in enumerate(groups):
                banks = {}
                for tl in group:
                    for n in range(2):
                        banks[(tl, n)] = ps3p.tile(
                            [P, TCH], dt32, name=f"ps3_{ch}_{tl}_{n}", tag="ps3"
                        )
                for e in range(E):
                    if e == 0 and w2_next[0] is not None:
                        w2t = w2_next[0]
                        w2_next[0] = None
                    else:
                        w2t = fetch_w2(e)
                    for tl in group:
                        toff = tl * P
                        for j in range(F1T // 2):
                            for n in range(2):
                                nc.tensor.matmul(
                                    banks[(tl, n)][:, :],
                                    h1a[:, e * F1T + 2 * j : e * F1T + 2 * j + 2,
                                        toff : toff + P],
                                    w2t[:, 2 * j : 2 * j + 2, n * TCH : (n + 1) * TCH],
                                    start=(e == 0 and j == 0),
                                    stop=(e == E - 1 and j == F1T // 2 - 1),
                                    perf_mode=DR,
                                )
                # prefetch the next group's first W2f before the drains
                if not (last and gi == len(groups) - 1):
                    w2_next[0] = fetch_w2(0)
                for tl in group:
                    tau = ch * 4 + tl
                    f2 = f2p.tile([P, H], dt32, name=f"f2_{tau}", tag="f2")
                    for n in range(2):
                        nc.vector.scalar_tensor_tensor(
                            f2[:, n * TCH : (n + 1) * TCH],
                            banks[(tl, n)][:, :],
                            hscl[:, 0:1],
                            xrfs[tau][:, n * TCH : (n + 1) * TCH],
                            op0=ALU.mult, op1=ALU.add,
                        )
                    emit_tail(tau, f2)

        # ---------- main schedule ----------
        # prefetch chunk-1 W1 for the first experts so the next mm1 phase
        # starts without waiting on the DMA queue
        w1_cache[(1, 0)] = fetch_w1(1, 0)
        ps3p = tc.alloc_tile_pool(name="ps3p", bufs=6, space="PSUM")
        proj_phase(0, h1a0, [(0, 1), (2, 3)])
        h1a1 = mm1_phase(1)
        proj_phase(1, h1a1, [(0, 1), (2, 3)], last=True)

        ps3p.release()
        sqp.release()
        otp.release()
        tp.release()
        f2p.release()
        txf.release()


def _prep_inputs(inputs):
    """Host-side sharding/layout prep. Returns per-core input maps."""
    f32 = np.float32

    def bf(x):
        return np.ascontiguousarray(np.asarray(x, dtype=f32)).astype(BF16)

    hs = np.ascontiguousarray(np.asarray(inputs["hidden_states"], dtype=f32))  # [B,S,H]

    e1_w = np.asarray(inputs["e1_w"], f32)          # [E, H, F1]
    e1_b = np.asarray(inputs["e1_b"], f32)          # [E, F1]
    e2_w = np.asarray(inputs["e2_w"], f32)          # [E, F1, H]
    e2_b = np.asarray(inputs["e2_b"], f32)          # [E, H]
    f_w = np.asarray(inputs["f_w"], f32)            # [E, H, H]

    # merge: W2f = W2 @ Wf   [E, F1, H];   b2f = b2 @ Wf   [E, H]
    w2f = np.matmul(e2_w, f_w)
    b2f = np.einsum("eh,ehg->eg", e2_b, f_w)

    # pooled mean over the sequence (gate MLP input), feature-major
    pooled = hs.mean(axis=1)                        # [B, H]
    pooled_t = np.ascontiguousarray(
        pooled.reshape(B, HT, P).transpose(2, 1, 0)
    ).reshape(P, HT * B)

    common = {
        "w1": (e1_w * WSCALE).astype(FP8).reshape(E, HT, P, F1),
        "w2f": (w2f * WSCALE).astype(FP8).reshape(E, F1T, P, H),
        "b1d": np.ascontiguousarray(e1_b.reshape(E, F1T, P).transpose(2, 0, 1)).reshape(P, E * F1T),
        "b2fd": bf(b2f),
        "pooled_d": pooled_t.astype(BF16),
        "a1": bf(inputs["a1_w"]).reshape(HT, P, F1),
        "a2": bf(inputs["a2_w"]).reshape(4, P, 256),
        "a3": bf(inputs["a3_w"]).reshape(2, P, 128),
        "s1": bf(inputs["s1_w"]),
        "s2": bf(inputs["s2_w"]),
        "s3": bf(inputs["s3_w"]),
        "ab1": np.ascontiguousarray(np.asarray(inputs["a1_b"], f32).reshape(4, P).T),
        "ab2": np.ascontiguousarray(np.asarray(inputs["a2_b"], f32).reshape(2, P).T),
        "ab3": np.ascontiguousarray(np.asarray(inputs["a3_b"], f32).reshape(1, P).T),
        "sb1": np.ascontiguousarray(np.asarray(inputs["s1_b"], f32).reshape(64, 1)),
        "sb2": np.ascontiguousarray(np.asarray(inputs["s2_b"], f32).reshape(32, 1)),
        "sb3": np.ascontiguousarray(np.broadcast_to(np.asarray(inputs["s3_b"], f32), (B, E))),
        "id128": np.eye(P, dtype=f32),
        "fbbc_d": np.ascontiguousarray(np.broadcast_to(np.asarray(inputs["f_b"], f32), (P, H))),
        "gbc_d": np.ascontiguousarray(np.broadcast_to(np.asarray(inputs["ln_g"], f32), (P, H))),
        "bbc_d": np.ascontiguousarray(np.broadcast_to(np.asarray(inputs["ln_b"], f32), (P, H))),
    }

    in_maps = []
    for c in range(NCORES):
        shard = hs[:, c * SC : (c + 1) * SC, :]                     # [B, SC, H]
        xsh_c = np.ascontiguousarray(shard.transpose(2, 0, 1)).reshape(H, TPC)
        m = dict(common)
        m["xsh"] = xsh_c.astype(FP8).reshape(HT, P, TPC)
        m["xres"] = np.ascontiguousarray(shard).reshape(TPC, H)
        in_maps.append(m)
    return in_maps


def kernel(**inputs) -> np.ndarray:
    nc = _build()
    in_maps = _prep_inputs(inputs)
    res = bass_utils.run_bass_kernel_spmd(nc, in_maps, core_ids=list(range(NCORES)))
    out_full = np.empty((B, S, H), dtype=np.float32)
    for c in range(NCORES):
        out_full[:, c * SC : (c + 1) * SC, :] = res.results[c]["out"].reshape(B, SC, H)
    return out_full
